# revision 16
# baseline (speedup 1.0000x reference)
"""AttentionBlock (GroupNorm + single-head full attention + residual) on 8
Trainium2 NeuronCores.

Sharding: data-parallel over batch (4) x sequence-parallel over query
tokens (2 halves of h*w=4096). Each core gets its batch slice with the
token axis ROTATED by the host so that its 2048 queries are always
columns 0:NQ (attention is permutation-invariant over keys, GroupNorm
over positions), so a single xb input serves stats, K, V and the query
slice. No collectives; the host scatters inputs and gathers outputs.

Per-core pipeline v2 (all-fp8-DoubleRow matmuls, no f32r anywhere):
 - x arrives as plain f32 over the HWDGE (sync/scalar) DMA queues --
   the v1 f32r-casting DMAs needed the gpsimd SWDGE queue whose ring
   boot delayed transfers ~9.5us. x is cast on-chip to fp8e4m3
   xf8 [128, 2(c-half), n] (ACT/DVE split, per chunk as DMA lands).
 - GroupNorm stats via DVE bn_stats/bn_aggr on the f32 x (single
   pass); group reduce/broadcast via tiny indicator matmuls (plain
   f32; PE is idle during the head anyway).
 - Normalization FOLDED INTO WEIGHTS: W' = W * a per input channel
   (a = gamma*rstd), bias' = W @ b + bias (b = beta - mean*a), so all
   projections consume RAW x. out_w folded into V: W2 = out_w @ Wv.
   Folded weights are written directly as fp8 [128, 2(c_in), out].
 - Q/K/V2 projections as fp8 DoubleRow matmuls (contraction 256 in
   one pass). Epilogue copies batch 2 (q/k) or 4 (v2) output chunks
   per multi-bank PSUM tile into ONE ACT/DVE op each (ACT reads PSUM
   at full rate, DVE at half -- ACT gets the bigger share).
 - Attention per 512-query chunk, per kt-PAIR: S^T = kf8-pair.T @
   qf8-pair (1 DR MM per kt); both S tiles of a pair land in one
   2-bank PSUM tile so a single ACT Exp (N=1024) writes pT fp8
   [128,2,512]. PV swaps operands vs v1: stationary = v2f8 kt-pair
   c-slice so the moving operand is pT (512 cols; LDWEIGHTS hides).
   po[c,q] needs no transpose. Softmax denominator via a third PV MM
   with an all-ones stationary -> po_l[q] replicated on all 128
   partitions, so 1/l is a plain elementwise multiply.
 - Epilogue per chunk: rl = reciprocal_approx_fast(po_l) (5x faster
   than DVE reciprocal); y = (x + ob_f) + po*rl via tensor_tensor +
   scalar_tensor_tensor; DMA out.
 - PE warmup bursts (early + late) during the DMA/stats wait so the
   HAM clock-gate is at 2.4 GHz when projections start.
 - exp(s/16 - 2): keeps P below fp8e4m3's 448 max for outlier scores
   (e4m3 overflow is NaN); the bias cancels exactly in the softmax.

Toolchain notes: this walrus accepts at most one sync-wait per
instruction (SplitWaitTileContext splits the rest onto nops).
"""

import numpy as np

B, C, HW = 4, 256, 4096
NQ = HW // 2
G = 8
CPG = C // G  # channels per group
EPS = 1e-5
N_CORES = 8
EXP_BIAS = -3.5

_CACHE = {}


def _split_wait_tc():
    import bass_rust
    import concourse.mybir as mybir
    import concourse.tile as tile
    from concourse.vector_clock import ScopedClock

    MAXW = 1

    class SplitWaitTileContext(tile.TileContext):
        """Workaround: this toolchain's walrus accepts at most one sync-wait
        per instruction; split excess waits onto same-engine InstNoOps."""

        def _split_excess_waits(self, inst):
            si = inst.sync_info
            if si is None:
                return []
            waits = list(si.on_wait)
            if len(waits) <= MAXW:
                return []
            extra, keep = waits[:-MAXW], waits[-MAXW:]
            nops = [
                mybir.InstNoOp(
                    name=f"I-{self.nc.next_id()}",
                    sync_info=mybir.SyncInfo(on_wait=[w], on_update=[]),
                    bass_nofuse=True,
                    engine=inst.engine,
                )
                for w in extra
            ]
            inst.sync_info = mybir.SyncInfo(on_wait=keep, on_update=list(si.on_update))
            return nops

        def _commit_and_lower(self, inst, original_block, old_bb_map, bb_to_exit_bb):
            for nop in self._split_excess_waits(inst):
                self._commit_instruction(nop, lazy_reg_writes=False)
            return super()._commit_and_lower(
                inst, original_block, old_bb_map, bb_to_exit_bb
            )

        def _drain_and_barrier(self, tick_clock, wait_clock):
            drain_inst = self.nc.sync.drain()
            wait_clock.add_sem_waits(
                drain_inst.ins, ScopedClock({None: tick_clock.global_clock})
            )
            si = drain_inst.ins.sync_info
            waits = list(si.on_wait) if si is not None else []
            if len(waits) > MAXW:
                updates = list(si.on_update) if si is not None else []
                drain_inst.ins.sync_info = bass_rust.SyncInfo(
                    on_wait=waits[:MAXW], on_update=[]
                )
                rest = waits[MAXW:]
                for i, w in enumerate(rest):
                    extra = self.nc.sync.drain()
                    extra.ins.sync_info = bass_rust.SyncInfo(
                        on_wait=[w], on_update=updates if i == len(rest) - 1 else []
                    )
            self.nc.all_engine_barrier()
            assert self.sems is not None
            popped = self.nc._tile_sem_poison_stack.pop()
            assert popped is self._sem_poison
            self.nc.clear_and_free_semaphores(list(self.sems.allocated().values()))
            self.nc.all_engine_barrier()

    return SplitWaitTileContext


def _build_nc(loop_reps=1, debug=False):
    import concourse.bass as bass
    import concourse.mybir as mybir

    F32 = mybir.dt.float32
    F8 = mybir.dt.float8e4
    AF = mybir.ActivationFunctionType
    ALU = mybir.AluOpType
    DR = mybir.MatmulPerfMode.DoubleRow

    SplitWaitTileContext = _split_wait_tc()

    nc = bass.Bass()
    xb = nc.dram_tensor("xb", [C, HW], F32, kind="ExternalInput")
    qkv_w = nc.dram_tensor("qkv_w", [3 * C, C], F32, kind="ExternalInput")
    qkv_b = nc.dram_tensor("qkv_b", [3 * C], F32, kind="ExternalInput")
    out_w = nc.dram_tensor("out_w", [C, C], F32, kind="ExternalInput")
    out_b = nc.dram_tensor("out_b", [C], F32, kind="ExternalInput")
    gn_gamma = nc.dram_tensor("gn_gamma", [C], F32, kind="ExternalInput")
    gn_beta = nc.dram_tensor("gn_beta", [C], F32, kind="ExternalInput")
    gind_in = nc.dram_tensor("gind_in", [128, 16], F32, kind="ExternalInput")
    hind_in = nc.dram_tensor("hind_in", [8, 128 * 2], F32, kind="ExternalInput")
    ones_in = nc.dram_tensor("ones_in", [128, 256], F32, kind="ExternalInput")
    wqkT_in = nc.dram_tensor("wqkT_in", [C, 512], F32, kind="ExternalInput")
    owT_in = nc.dram_tensor("owT_in", [C, C], F32, kind="ExternalInput")
    y = nc.dram_tensor("y", [C, NQ], F32, kind="ExternalOutput")
    if debug:
        d_q = nc.dram_tensor("d_q", [2, 128, NQ], F32, kind="ExternalOutput")
        d_k = nc.dram_tensor("d_k", [2, 128, HW], F32, kind="ExternalOutput")
        d_v2 = nc.dram_tensor("d_v2", [128, 32, 256], F32, kind="ExternalOutput")
        d_pol = nc.dram_tensor("d_pol", [128, 512], F32, kind="ExternalOutput")

    with SplitWaitTileContext(nc) as tc:
        import contextlib

        ctx = contextlib.ExitStack()
        with ctx:
            singles = ctx.enter_context(tc.tile_pool(name="singles", bufs=1))
            xpool = ctx.enter_context(tc.tile_pool(name="xpool", bufs=2))
            x8pool = ctx.enter_context(tc.tile_pool(name="x8pool", bufs=2))
            qpool = ctx.enter_context(tc.tile_pool(name="qpool", bufs=2))
            kpool = ctx.enter_context(tc.tile_pool(name="kpool", bufs=2))
            vpool = ctx.enter_context(tc.tile_pool(name="vpool", bufs=2))
            ypool = ctx.enter_context(tc.tile_pool(name="ypool", bufs=2))
            wpool = ctx.enter_context(tc.tile_pool(name="wpool", bufs=1))
            ppool = ctx.enter_context(tc.tile_pool(name="ppool", bufs=3))
            opool = ctx.enter_context(tc.tile_pool(name="opool", bufs=2))
            stat = ctx.enter_context(tc.tile_pool(name="stat", bufs=2))
            if debug:
                dbgp = ctx.enter_context(tc.tile_pool(name="dbgp", bufs=1))
            psmm = ctx.enter_context(tc.tile_pool(name="psmm", bufs=2, space="PSUM"))
            psov = ctx.enter_context(tc.tile_pool(name="psov", bufs=1, space="PSUM"))
            pssm = ctx.enter_context(tc.tile_pool(name="pssm", bufs=1, space="PSUM"))

            def setup():
                eps_sb = singles.tile([8, 1], F32, tag="eps")
                nc.vector.memset(eps_sb, EPS)
                ebias_sb = singles.tile([128, 1], F32, tag="ebias")
                nc.vector.memset(ebias_sb, EXP_BIAS)
                gam_sb = singles.tile([128, 2], F32, tag="gam")
                bet_sb = singles.tile([128, 2], F32, tag="bet")
                qb_sb = singles.tile([128, 6], F32, tag="qb")
                ob_sb = singles.tile([128, 2], F32, tag="ob")
                gi_sb = singles.tile([128, 16], F32, tag="gi")
                hi_sb = singles.tile([8, 128 * 2], F32, tag="hi")
                ones_sb = singles.tile([128, 256], F32, tag="ones32")
                ones_f8 = singles.tile([128, 2, 128], F8, tag="ones8")

                def load_consts():
                    nc.sync.dma_start(out=qb_sb, in_=qkv_b.rearrange("(m p) -> p m", p=128))
                    nc.sync.dma_start(out=gam_sb, in_=gn_gamma.rearrange("(t p) -> p t", p=128))
                    nc.sync.dma_start(out=bet_sb, in_=gn_beta.rearrange("(t p) -> p t", p=128))
                    nc.sync.dma_start(out=ob_sb, in_=out_b.rearrange("(t p) -> p t", p=128))
                    nc.sync.dma_start(out=gi_sb, in_=gind_in[:, :])
                    nc.sync.dma_start(out=hi_sb, in_=hind_in[:, :])
                    nc.sync.dma_start(out=ones_sb, in_=ones_in[:, :])
                    nc.vector.tensor_copy(
                        ones_f8.rearrange("p a b -> p (a b)"), ones_sb
                    )

                g_sb = [gam_sb[:, t : t + 1] for t in range(2)]
                be_sb = [bet_sb[:, t : t + 1] for t in range(2)]
                gind = [gi_sb[:, 0:8], gi_sb[:, 8:16]]
                hind = [hi_sb[:, 0:128], hi_sb[:, 128:256]]
                return (g_sb, be_sb, qb_sb, ob_sb, eps_sb, ebias_sb, gind, hind,
                        ones_f8, load_consts)

            def body(rep, consts):
                (g_sb, be_sb, qb_sb, ob_sb, eps_sb, ebias_sb, gind, hind,
                 ones_f8, load_consts) = consts
                # x as plain f32 over the HWDGE queues (sync+scalar), 4KB
                # lines, 8 chunks interleaved across both engines.
                x_sb = []
                for t in range(2):
                    xt = xpool.tile([128, HW], F32, tag="xv", name=f"x{t}")
                    for c4 in range(4):
                        eng = nc.sync if (t * 4 + c4) % 2 == 0 else nc.scalar
                        eng.dma_start(
                            out=xt[:, c4 * 1024 : (c4 + 1) * 1024],
                            in_=xb[t * 128 : (t + 1) * 128, c4 * 1024 : (c4 + 1) * 1024],
                        )
                    x_sb.append(xt)
                # host pre-rotates xb per core so the query half is always
                # columns 0:NQ (attention is permutation-invariant over keys)
                xq_sb = [x_sb[t][:, 0:NQ] for t in range(2)]

                # weights arrive pre-transposed from the host
                wT = []  # (Wq|Wk)^T tiles [c_in 128, 512] f32
                for t in range(2):
                    wT.append(wpool.tile([128, 512], F32, tag=f"wT{t}", name=f"wTn{t}"))
                owT = []  # out_w^T tiles [c_in 128, 256] f32
                for t in range(2):
                    owT.append(wpool.tile([128, 256], F32, tag=f"owT{t}", name=f"owT{t}"))
                for t in range(2):
                    nc.sync.dma_start(out=wT[t], in_=wqkT_in[t * 128 : (t + 1) * 128, :])
                    nc.scalar.dma_start(out=owT[t], in_=owT_in[t * 128 : (t + 1) * 128, :])
                load_consts()

                # ---------- W2 = out_w @ Wv fold (plain f32; head slack) ----------
                wv_f = []
                for i in range(2):
                    wv = wpool.tile([128, C], F32, tag=f"wv{i}", name=f"wv{i}")
                    nc.scalar.dma_start(
                        out=wv, in_=qkv_w[512 + i * 128 : 512 + (i + 1) * 128, :]
                    )
                    wv_f.append(wv)
                w2t = []
                for t in range(2):
                    ps = pssm.tile([128, 512], F32, tag="sm", name=f"w2f{t}")
                    nc.tensor.matmul(
                        ps[:, 0:256], wv_f[0][:, t * 128 : (t + 1) * 128], owT[0],
                        start=True, stop=False,
                    )
                    nc.tensor.matmul(
                        ps[:, 0:256], wv_f[1][:, t * 128 : (t + 1) * 128], owT[1],
                        start=False, stop=True,
                    )
                    w2 = wpool.tile([128, 256], F32, tag=f"w2t{t}", name=f"w2t{t}")
                    nc.vector.tensor_copy(w2, ps[:, 0:256])
                    w2t.append(w2)
                # ob_eff = out_b + out_w @ bv  (bv = qkv_b[512:768])
                ps_ob = pssm.tile([128, 512], F32, tag="sm", name="ps_ob")
                for m2 in range(2):
                    nc.tensor.matmul(
                        ps_ob[:, m2 : m2 + 1],
                        owT[0][:, m2 * 128 : (m2 + 1) * 128],
                        qb_sb[:, 4:5],
                        start=True, stop=False,
                    )
                    nc.tensor.matmul(
                        ps_ob[:, m2 : m2 + 1],
                        owT[1][:, m2 * 128 : (m2 + 1) * 128],
                        qb_sb[:, 5:6],
                        start=False, stop=True,
                    )
                ob_eff = stat.tile([128, 2], F32, tag="obeff")
                nc.vector.tensor_add(ob_eff, ps_ob[:, 0:2], ob_sb)

                # ---------- PE warmup (early burst on x chunk 0) ----------
                for wi in range(6):
                    psw = pssm.tile([128, 512], F32, tag="sm", name=f"warm{wi}")
                    nc.tensor.matmul(
                        psw, wT[wi % 2][:, 0:128], x_sb[0][:, 0:512],
                        start=True, stop=True,
                    )

                # ---------- xf8 cast (chunked, as x DMA lands; idle GPSIMD) ----------
                # xf8[p, t, n] = fp8(x[t*128+p, n]) -- DoubleRow pair layout.
                xf8 = x8pool.tile([128, 2, HW], F8, tag="x8", name="xf8")
                for t in range(2):
                    for c4 in range(4):
                        sl = slice(c4 * 1024, (c4 + 1) * 1024)
                        nc.gpsimd.tensor_copy(xf8[:, t, sl], x_sb[t][:, sl])

                # ---------- GroupNorm stats (DVE bn_stats) ----------
                st2 = []
                for t in range(2):
                    bst = stat.tile([128, 8, 6], F32, tag=f"bst{t}", name=f"bst{t}")
                    for c8 in range(8):
                        nc.vector.bn_stats(
                            out=bst[:, c8, :],
                            in_=x_sb[t][:, c8 * 512 : (c8 + 1) * 512],
                        )
                    mv = stat.tile([128, 2], F32, tag=f"mv{t}", name=f"mv{t}")
                    nc.vector.bn_aggr(out=mv, in_=bst.rearrange("p a b -> p (a b)"))
                    # st2 = [mean_c, E[x^2]_c]
                    s2t = stat.tile([128, 2], F32, tag=f"st2{t}")
                    nc.vector.tensor_copy(s2t[:, 0:1], mv[:, 0:1])
                    nc.vector.scalar_tensor_tensor(
                        out=s2t[:, 1:2], in0=mv[:, 0:1], scalar=mv[:, 0:1],
                        in1=mv[:, 1:2], op0=ALU.mult, op1=ALU.add,
                    )
                    st2.append(s2t)

                # ---------- PE warmup (late top-up; depends on x t1 chunk 2
                # so it runs alongside the final DMA chunk) ----------
                for wi in range(4):
                    psw = pssm.tile([128, 512], F32, tag="sm", name=f"warml{wi}")
                    nc.tensor.matmul(
                        psw[:, 0:128], wT[wi % 2][:, 0:128],
                        x_sb[1][:, 2048 : 2048 + 128],
                        start=True, stop=True,
                    )

                psg = pssm.tile([128, 512], F32, tag="sm", name="psg")
                nc.tensor.matmul(psg[0:8, 0:2], gind[0], st2[0], start=True, stop=False)
                nc.tensor.matmul(psg[0:8, 0:2], gind[1], st2[1], start=False, stop=True)
                gstat = stat.tile([8, 2], F32, tag="gstat")  # [mean_g, E[x^2]_g]
                nc.vector.tensor_scalar_mul(gstat, psg[0:8, 0:2], 1.0 / CPG)
                var_g = stat.tile([8, 1], F32, tag="varg")
                nc.vector.tensor_mul(var_g, gstat[:, 0:1], gstat[:, 0:1])
                nc.vector.tensor_sub(var_g, gstat[:, 1:2], var_g)
                std_g = stat.tile([8, 1], F32, tag="stdg")
                nc.scalar.activation(out=std_g, in_=var_g, func=AF.Sqrt, bias=eps_sb, scale=1.0)
                rm = stat.tile([8, 2], F32, tag="rm")  # [rstd_g, mean_g]
                nc.vector.reciprocal(rm[:, 0:1], std_g)
                nc.vector.tensor_copy(rm[:, 1:2], gstat[:, 0:1])
                # broadcast to channels: [rstd_c, mean_c] = H_t.T @ rm
                ab = []
                for t in range(2):
                    psb = pssm.tile([128, 512], F32, tag="sm", name=f"psb{t}")
                    nc.tensor.matmul(psb[:, 0:2], hind[t], rm, start=True, stop=True)
                    abt = stat.tile([128, 2], F32, tag=f"ab{t}")  # [a_c, b_c]
                    nc.vector.tensor_mul(abt[:, 0:1], psb[:, 0:1], g_sb[t])
                    nc.vector.tensor_mul(abt[:, 1:2], psb[:, 1:2], abt[:, 0:1])
                    nc.vector.tensor_sub(abt[:, 1:2], be_sb[t], abt[:, 1:2])
                    ab.append(abt)

                # ---------- fold GN into weights (fp8 outputs) ----------
                # wf8[p, t, :] = fp8(wT[t] * a_c);  w2f8[p, t, :] = fp8(w2t[t] * a_c)
                wf8 = wpool.tile([128, 2, 512], F8, tag="wf8", name="wf8")
                w2f8 = wpool.tile([128, 2, 256], F8, tag="w2f8", name="w2f8")
                for t in range(2):
                    nc.vector.tensor_scalar_mul(wf8[:, t, :], wT[t], ab[t][:, 0:1])
                    nc.vector.tensor_scalar_mul(w2f8[:, t, :], w2t[t], ab[t][:, 0:1])
                ps_qb = pssm.tile([128, 512], F32, tag="sm", name="ps_qb")
                for m in range(4):
                    nc.tensor.matmul(
                        ps_qb[:, m : m + 1],
                        wT[0][:, m * 128 : (m + 1) * 128],
                        ab[0][:, 1:2],
                        start=True, stop=False,
                    )
                    nc.tensor.matmul(
                        ps_qb[:, m : m + 1],
                        wT[1][:, m * 128 : (m + 1) * 128],
                        ab[1][:, 1:2],
                        start=False, stop=True,
                    )
                qb_eff = stat.tile([128, 4], F32, tag="qbeff")
                nc.vector.tensor_add(qb_eff, ps_qb[:, 0:4], qb_sb[:, 0:4])
                ps_ob2 = pssm.tile([128, 512], F32, tag="sm", name="ps_ob2")
                for m2 in range(2):
                    nc.tensor.matmul(
                        ps_ob2[:, m2 : m2 + 1],
                        w2t[0][:, m2 * 128 : (m2 + 1) * 128],
                        ab[0][:, 1:2],
                        start=True, stop=False,
                    )
                    nc.tensor.matmul(
                        ps_ob2[:, m2 : m2 + 1],
                        w2t[1][:, m2 * 128 : (m2 + 1) * 128],
                        ab[1][:, 1:2],
                        start=False, stop=True,
                    )
                ob_f = stat.tile([128, 2], F32, tag="obf")
                nc.vector.tensor_add(ob_f, ps_ob2[:, 0:2], ob_eff)

                # ---------- qkv projections (fp8 DR; 2 chunks per PSUM tile) ----------
                qf8 = qpool.tile([128, 2, NQ], F8, tag="q", name="qf8")
                kf8 = kpool.tile([128, 2, HW], F8, tag="k", name="kf8")
                nch = 0
                for m in (2, 3, 0, 1):
                    t_half = m % 2
                    dst = qf8 if m < 2 else kf8
                    n_all = NQ if m < 2 else HW
                    nj = n_all // 512
                    ps_t = None
                    for j in range(nj):
                        if j % 2 == 0:
                            ps_t = psmm.tile([128, 1024], F32, tag="spair", name="pj")
                        ps = ps_t[:, (j % 2) * 512 : (j % 2 + 1) * 512]
                        nc.tensor.matmul(
                            ps,
                            wf8[:, :, m * 128 : (m + 1) * 128],
                            xf8[:, :, j * 512 : (j + 1) * 512],
                            start=True, stop=True,
                            perf_mode=DR, skip_group_check=True,
                        )
                        if j % 2 == 1:
                            dsl = dst[:, t_half, (j - 1) * 512 : (j + 1) * 512]
                            if nch % 3 != 0:
                                nc.scalar.activation(
                                    out=dsl, in_=ps_t, func=AF.Identity,
                                    bias=qb_eff[:, m : m + 1], scale=1.0,
                                )
                            else:
                                nc.vector.tensor_scalar_add(
                                    dsl, ps_t, qb_eff[:, m : m + 1]
                                )
                            nch += 1

                # ---------- V2 = x^T @ W2' -> fp8 [128, kt, 256] ----------
                v2f8 = vpool.tile([128, 32, 256], F8, tag="v2", name="v2f8")
                ps_t = None
                for nt in range(32):
                    if nt % 4 == 0:
                        ps_t = psmm.tile([128, 1024], F32, tag="spair", name="pv")
                    ps = ps_t[:, (nt % 4) * 256 : (nt % 4 + 1) * 256]
                    nc.tensor.matmul(
                        ps,
                        xf8[:, :, nt * 128 : (nt + 1) * 128],
                        w2f8,
                        start=True, stop=True,
                        perf_mode=DR, skip_group_check=True,
                    )
                    if nt % 4 == 3:
                        dsl = v2f8[:, nt - 3 : nt + 1, :]
                        if (nt // 4) % 3 != 0:
                            nc.scalar.copy(dsl, ps_t.rearrange("p (a b) -> p a b", b=256))
                        else:
                            nc.vector.tensor_copy(
                                dsl, ps_t.rearrange("p (a b) -> p a b", b=256)
                            )

                if debug:
                    for t in range(2):
                        dq = dbgp.tile([128, NQ], F32, tag="dq", name=f"dq{t}")
                        nc.vector.tensor_copy(dq, qf8[:, t, :])
                        nc.sync.dma_start(out=d_q[t, :, :], in_=dq)
                        dk = dbgp.tile([128, HW], F32, tag="dk", name=f"dk{t}")
                        nc.vector.tensor_copy(dk, kf8[:, t, :])
                        nc.sync.dma_start(out=d_k[t, :, :], in_=dk)
                    dv = dbgp.tile([128, 32, 256], F32, tag="dv", name="dv")
                    nc.vector.tensor_copy(dv, v2f8)
                    nc.sync.dma_start(out=d_v2[:, :, :], in_=dv)

                # ---------- attention ----------
                y_sb = [
                    ypool.tile([128, NQ], F32, tag="y", name=f"y{t}") for t in range(2)
                ]
                for qc in range(NQ // 512):
                    qsl = qf8[:, :, qc * 512 : (qc + 1) * 512]
                    po_A = psov.tile([128, 512], F32, tag="poA", name="poA")
                    po_B = psov.tile([128, 512], F32, tag="poB", name="poB")
                    # alternate the l bank with the head-phase "sm" bank so
                    # the slow DVE reciprocal never blocks the next chunk's
                    # PV accumulation (2 chunks of slack per bank).
                    if qc % 2 == 0:
                        po_l = psov.tile([128, 512], F32, tag="pol", name="pol")
                    else:
                        po_l = pssm.tile([128, 512], F32, tag="sm", name="pol2")
                    for j in range(16):
                        ps = psmm.tile([128, 1024], F32, tag="spair", name="spair")
                        for i in range(2):
                            kt = 2 * j + i
                            nc.tensor.matmul(
                                ps[:, i * 512 : (i + 1) * 512],
                                kf8[:, :, kt * 128 : (kt + 1) * 128],
                                qsl,
                                start=True, stop=True,
                                perf_mode=DR,
                                skip_group_check=True,
                            )
                        pT = ppool.tile([128, 2, 512], F8, tag="p", name="pT")
                        nc.scalar.activation(
                            out=pT,
                            in_=ps.rearrange("p (a b) -> p a b", b=512),
                            func=AF.Exp,
                            scale=1.0 / 16.0,
                            bias=ebias_sb,
                        )
                        vsl = v2f8[:, 2 * j : 2 * j + 2, :]
                        nc.tensor.matmul(
                            po_A, vsl[:, :, 0:128], pT,
                            start=(j == 0), stop=(j == 15),
                            perf_mode=DR, skip_group_check=True,
                        )
                        nc.tensor.matmul(
                            po_B, vsl[:, :, 128:256], pT,
                            start=(j == 0), stop=(j == 15),
                            perf_mode=DR, skip_group_check=True,
                        )
                        nc.tensor.matmul(
                            po_l, ones_f8, pT,
                            start=(j == 0), stop=(j == 15),
                            perf_mode=DR, skip_group_check=True,
                        )
                    if debug and qc == 0:
                        dpl = dbgp.tile([128, 512], F32, tag="dpl", name="dpl")
                        nc.vector.tensor_copy(dpl, po_l)
                        nc.sync.dma_start(out=d_pol[:, :], in_=dpl)
                    # epilogue: y = (x + ob_f) + po * (1/l).  Evacuate the
                    # A/B banks immediately (ACT reads PSUM at full rate) so
                    # the next chunk's PV matmuls never stall; the reciprocal
                    # and scaling then run SBUF-side in the next chunk's
                    # shadow.
                    oA = opool.tile([128, 512], F32, tag="oA", name="oA")
                    nc.scalar.copy(oA, po_A)
                    oB = opool.tile([128, 512], F32, tag="oB", name="oB")
                    nc.vector.tensor_copy(oB, po_B)
                    rl = opool.tile([128, 512], F32, tag="rl", name="rl")
                    nc.vector.reciprocal(rl, po_l)
                    tmp = [
                        opool.tile([128, 512], F32, tag=f"tm{t}", name=f"tm{t}")
                        for t in range(2)
                    ]
                    nc.vector.tensor_mul(tmp[0], oA, rl)
                    nc.vector.tensor_mul(tmp[1], oB, rl)
                    for t in range(2):
                        ysl = y_sb[t][:, qc * 512 : (qc + 1) * 512]
                        nc.vector.scalar_tensor_tensor(
                            out=ysl,
                            in0=xq_sb[t][:, qc * 512 : (qc + 1) * 512],
                            scalar=ob_f[:, t : t + 1],
                            in1=tmp[t],
                            op0=ALU.add,
                            op1=ALU.add,
                        )
                        nc.sync.dma_start(
                            out=y[t * 128 : (t + 1) * 128, qc * 512 : (qc + 1) * 512],
                            in_=ysl,
                        )

            consts = setup()
            for rep in range(loop_reps):
                body(rep, consts)

    return nc


def _get_runner(loop_reps=1):
    key = ("runner", loop_reps)
    if key not in _CACHE:
        nc = _build_nc(loop_reps)
        _CACHE[key] = nc
    return _CACHE[key]


def make_extra_inputs():
    gind = np.zeros((128, 16), dtype=np.float32)
    hind = np.zeros((8, 256), dtype=np.float32)
    for t in range(2):
        for p in range(128):
            g = (t * 128 + p) // CPG
            gind[p, t * 8 + g] = 1.0
            hind[g, t * 128 + p] = 1.0
    return {"gind_in": gind, "hind_in": hind,
            "ones_in": np.ones((128, 256), dtype=np.float32)}


def make_weight_inputs(qkv_w, out_w):
    return {
        "wqkT_in": np.ascontiguousarray(qkv_w[0:512].T),
        "owT_in": np.ascontiguousarray(out_w.T),
    }


def kernel(x, gn_gamma, gn_beta, qkv_w, qkv_b, out_w, out_b):
    from concourse.bass_utils import run_bass_kernel_spmd

    x = np.asarray(x, dtype=np.float32)
    gn_gamma = np.asarray(gn_gamma, dtype=np.float32)
    gn_beta = np.asarray(gn_beta, dtype=np.float32)
    qkv_w = np.asarray(qkv_w, dtype=np.float32)
    qkv_b = np.asarray(qkv_b, dtype=np.float32)
    out_w = np.asarray(out_w, dtype=np.float32)
    out_b = np.asarray(out_b, dtype=np.float32)

    b, c, h, w = x.shape
    assert (b, c, h * w) == (B, C, HW)
    xf = x.reshape(b, c, HW)

    nc = _get_runner()
    in_maps = []
    for j in range(N_CORES):
        bi, qh = j // 2, j % 2
        if qh == 0:
            xbj = np.ascontiguousarray(xf[bi])
        else:
            xbj = np.concatenate([xf[bi][:, NQ:], xf[bi][:, :NQ]], axis=1)
        in_maps.append(
            {
                "xb": xbj,
                "qkv_w": qkv_w,
                "qkv_b": qkv_b,
                "out_w": out_w,
                "out_b": out_b,
                "gn_gamma": gn_gamma,
                "gn_beta": gn_beta,
            }
        )
    extras = make_extra_inputs()
    extras.update(make_weight_inputs(qkv_w, out_w))
    for m in in_maps:
        m.update(extras)
    res = run_bass_kernel_spmd(nc, in_maps, core_ids=list(range(N_CORES)))
    out = np.empty((B, C, HW), dtype=np.float32)
    for j in range(N_CORES):
        bi, qh = j // 2, j % 2
        out[bi][:, qh * NQ : (qh + 1) * NQ] = res.results[j]["y"]
    return out.reshape(b, c, h, w)
